# revision 17
# baseline (speedup 1.0000x reference)
"""Trainium2 Bass kernel for nn_DifferentiableCDF (soft Gaussian histogram -> CDF).

Algorithm change vs the soft-binning baseline: the Gaussian soft-binning weight
exp(-(255x - j)^2 / 2.55^2) depends only on (u - j) with u = 255x, and each
pixel's total mass sum_j g(j - u) is (away from the [0,255] edges) a constant
independent of u.  Quantizing u -> m = round(u) therefore preserves per-pixel
mass exactly and perturbs the per-bin histogram only by ~N(0, h^2/12 * sum g'^2)
≈ 0.06% relative (verified 1.6e-4 end-to-end CDF error vs the fp64 reference).

So the device only computes a 256-bin COUNT histogram per (B,C) unit, and the
host applies the exact Gaussian spreading as a 33-tap float64 convolution,
normalizes, and cumsums (same scale of host work as the baseline's fold).

Device per core (98304 px as 768 chunks of 128):
  m = RNE(255x) in [0,255]; J = m>>4; r = m&15 (exact fp tricks).
  DVE builds 16-wide one-hots of J and r (bf16 4x tensor_scalar is_equal).
  TensorE: 8 chunks are packed per matmul: stationary = ohJ of 8 chunks side
  by side [128, 128] (full-width -> Fast Weight Load), moving = ohr of the
  same 8 chunks [128, 128]; out[8J+i, 8r+i'] accumulates in PSUM.  Only the
  slot-diagonal i=i' sub-blocks are meaningful (extracted on host); packing
  cuts 768 small matmuls down to 96 full-width ones.
"""
import sys
if "/opt/trn_rl_repo" not in sys.path:
    sys.path.insert(0, "/opt/trn_rl_repo")

import numpy as np
from concourse import bacc, tile
from concourse.bass_utils import run_bass_kernel_spmd
import concourse.mybir as mybir

# ---- problem constants (hardcoded per spec) ----
B, C, H, W = 4, 3, 256, 256
UNITS = B * C                  # 12 independent histograms
NPIX = H * W                   # 65536 pixels per unit
NCORES = 8
PIX_PER_CORE = NPIX // NCORES  # 8192 pixels per unit per core
CHUNKS_PER_UNIT = PIX_PER_CORE // 128  # 64
NCHUNK = UNITS * CHUNKS_PER_UNIT       # 768 chunks of 128 pixels
SIGMA = 0.01
BINS = 256
SIG_B = 255.0 * SIGMA                  # 2.55 bins: gaussian width in bin units
KTAP = 16                              # host conv halfwidth (g(16/2.55) ~ 6e-18)
NGROUP = 3                             # pipeline groups (4 units each)
GCHUNK = NCHUNK // NGROUP              # 256 chunks per group
PACK = 8                               # chunks per matmul
DT = mybir.dt

_COMPILED = None


def _emit_body(nc, tc, pool, pipe, psum_pool, x_ext, tbl_ext,
               emit_cols=True, emit_mm=True):
    xc = pool.tile([128, NCHUNK], DT.float32)
    nc.sync.dma_start(xc[:], x_ext[:])

    # slot-major content: column i*96 + gp holds chunk (slot i, global pack gp)
    # int16 chain: 2-byte dtypes keep the DVE fast modes (fp32 ops run 1x).
    m_s = pool.tile([128, NCHUNK], DT.int16)
    J_s = pool.tile([128, NCHUNK], DT.int16)
    r_s = pool.tile([128, NCHUNK], DT.int16)

    # m = RNE(255*x) via int16-convert; exact (255x <= 255).
    nc.vector.tensor_scalar(m_s[:], xc[:], 255.0, None, mybir.AluOpType.mult)
    # J = floor(m/16) = RNE(m/16 - 15/32): m/16 lies on a 1/16 grid, so the
    # offset keeps every value >= 1/32 away from a rounding boundary.
    nc.vector.tensor_scalar(J_s[:], m_s[:], 0.0625, -0.46875,
                            mybir.AluOpType.mult, mybir.AluOpType.add)
    # r = m - 16J in [0,16)
    nc.vector.scalar_tensor_tensor(r_s[:], J_s[:], -16.0, m_s[:],
                                   mybir.AluOpType.mult, mybir.AluOpType.add)

    accs = [psum_pool.tile([128, 4 * PACK * 16], DT.float32, name=f"acc{g}")
            for g in range(NGROUP)] if emit_mm else None
    out_sb = pool.tile([128, NGROUP * 512], DT.float32)

    NPACKS = NCHUNK // PACK  # 96 global packs
    # one-hots in (v, slot, pack) layout: ohJ[:, v, i, gp] = [J == v] for
    # chunk (slot i, global pack gp).  Each is_equal reads/writes a fully
    # contiguous [128, 768] region -> DVE 4x mode (~240ns/op measured).  The
    # matmul stationary slice [:, :, :, gp] has free strides (768, 96) which
    # collapse to a single free dim of stride 96 (legal weights AP).
    ohJ = pipe.tile([128, 16, PACK, NPACKS], DT.bfloat16, tag="ohJ")
    ohr = pipe.tile([128, 16, PACK, NPACKS], DT.bfloat16, tag="ohr")
    FLAT = frozenset({0})  # collapse free dims -> flat AP so DVE 4x engages
    if emit_cols:
        for v in range(16):
            nc.vector.tensor_scalar(ohJ[:, v, :, :].opt(FLAT), J_s[:],
                                    float(v), None, mybir.AluOpType.is_equal)
            nc.vector.tensor_scalar(ohr[:, v, :, :].opt(FLAT), r_s[:],
                                    float(v), None, mybir.AluOpType.is_equal)
    npk = CHUNKS_PER_UNIT // PACK  # 8 packs per unit
    for g in range(NGROUP):
        if emit_mm:
            for uu in range(4):  # 4 units per group
                for q in range(npk):
                    gp = (g * 4 + uu) * npk + q
                    nc.tensor.matmul(accs[g][:, uu * 128:(uu + 1) * 128],
                                     ohJ[:, :, :, gp],
                                     ohr[:, :, :, gp],
                                     start=(q == 0), stop=(q == npk - 1))
            nc.scalar.copy(out_sb[:, g * 512:(g + 1) * 512], accs[g][:])
        else:
            # ablation: keep ACT-copy volume identical without reading PSUM
            nc.scalar.copy(out_sb[:, g * 512:(g + 1) * 512], xc[:, 0:512])
        nc.sync.dma_start(tbl_ext[:, g * 512:(g + 1) * 512],
                          out_sb[:, g * 512:(g + 1) * 512])


def _build(loop_n=1, emit_cols=True, emit_mm=True):
    nc = bacc.Bacc("TRN2", target_bir_lowering=False, debug=False,
                   num_devices=NCORES)
    x_ext = nc.declare_dram_parameter("xc", [128, NCHUNK], DT.float32,
                                      isOutput=False)
    tbl_ext = nc.declare_dram_parameter("table", [128, NGROUP * 512],
                                        DT.float32, isOutput=True)

    with tile.TileContext(nc) as tc:
        with (
            tc.tile_pool(name="pool", bufs=1) as pool,
            tc.tile_pool(name="pipe", bufs=2) as pipe,
            tc.tile_pool(name="psum", bufs=1, space="PSUM") as psum_pool,
        ):
            if loop_n == 1:
                _emit_body(nc, tc, pool, pipe, psum_pool, x_ext, tbl_ext,
                           emit_cols, emit_mm)
            else:
                engs = [mybir.EngineType.PE, mybir.EngineType.DVE,
                        mybir.EngineType.Activation, mybir.EngineType.SP,
                        mybir.EngineType.Pool]
                with tc.For_i(0, loop_n, 1, hint_engines=engs):
                    _emit_body(nc, tc, pool, pipe, psum_pool, x_ext, tbl_ext,
                               emit_cols, emit_mm)

    nc.compile()
    return nc


def _get_compiled():
    global _COMPILED
    if _COMPILED is None:
        _COMPILED = _build()
    return _COMPILED


def _shard_x(x):
    """x (B,C,H,W) -> per-core [128, NCHUNK] arrays in slot-major order:
    column i*96 + u*8 + q holds chunk (unit u, pack q, slot i), whose pixels
    are unit u's core-slice pixels [128*(8q+i) : 128*(8q+i+1)]."""
    xu = np.ascontiguousarray(x.reshape(UNITS, NPIX))
    shards = []
    for core in range(NCORES):
        sl = xu[:, core * PIX_PER_CORE:(core + 1) * PIX_PER_CORE]
        # (u, q, i, p) -> (p, i, u, q)
        sl = sl.reshape(UNITS, CHUNKS_PER_UNIT // PACK, PACK, 128)
        sl = sl.transpose(3, 2, 0, 1)
        shards.append(np.ascontiguousarray(sl.reshape(128, NCHUNK), np.float32))
    return shards


def _postprocess(tables):
    """tables: list of NCORES arrays [128, 1536] -> cdf (B, C, BINS) fp32."""
    cnt = np.zeros((UNITS, 16, 16), np.float64)   # [unit, J, r]
    for t in tables:
        # rows = (J:16, i:8); cols = (g:3, uu:4, r:16, i':8); diag i==i'
        t6 = t.reshape(16, 8, NGROUP, 4, 16, 8).astype(np.float64)
        cnt += np.einsum('jiguri->gujr', t6).reshape(UNITS, 16, 16)
    count = cnt.reshape(UNITS, BINS)              # bin m = 16J + r
    ks = np.arange(-KTAP, KTAP + 1)
    g = np.exp(-(ks / SIG_B) ** 2)
    hist = np.zeros((UNITS, BINS), np.float64)
    for i, k in enumerate(ks):
        lo, hi = max(0, k), min(BINS, BINS + k)
        hist[:, lo:hi] += g[i] * count[:, lo - k:hi - k]
    pdf = hist / (hist.sum(-1, keepdims=True) + 1e-6)
    cdf = np.cumsum(pdf, -1)
    return cdf.reshape(B, C, BINS).astype(np.float32)


def run_device(x, trace=False):
    nc = _get_compiled()
    in_maps = [{"xc": s} for s in _shard_x(np.asarray(x))]
    res = run_bass_kernel_spmd(nc, in_maps, list(range(NCORES)), trace=trace)
    tables = [res.results[i]["table"] for i in range(NCORES)]
    return tables, res


def kernel(x, centers):
    # centers is linspace(0,1,256) by construction; bin geometry is hardcoded.
    tables, _ = run_device(x)
    return _postprocess(tables)


if __name__ == "__main__":
    import jax, jax.numpy as jnp
    key = jax.random.key(0)
    k1, _ = jax.random.split(key)
    x = np.asarray(jax.random.uniform(k1, (B, C, H, W), dtype=jnp.float32))
    centers = np.linspace(0, 1, BINS, dtype=np.float32)
    out = kernel(x, centers)
    print("kernel output", out.shape, out.dtype, out[0, 0, :5], out[0, 0, -1])


# revision 18
# speedup vs baseline: 1.1914x; 1.1914x over previous
"""Trainium2 Bass kernel for nn_DifferentiableCDF (soft Gaussian histogram -> CDF).

Algorithm change vs the soft-binning baseline: the Gaussian soft-binning weight
exp(-(255x - j)^2 / 2.55^2) depends only on (u - j) with u = 255x, and each
pixel's total mass sum_j g(j - u) is (away from the [0,255] edges) a constant
independent of u.  Quantizing u -> m = round(u) therefore preserves per-pixel
mass exactly and perturbs the per-bin histogram only by ~N(0, h^2/12 * sum g'^2)
≈ 0.06% relative (verified 1.6e-4 end-to-end CDF error vs the fp64 reference).

So the device only computes a 256-bin COUNT histogram per (B,C) unit, and the
host applies the exact Gaussian spreading as a 33-tap float64 convolution,
normalizes, and cumsums (same scale of host work as the baseline's fold).

Device per core (98304 px as 768 chunks of 128):
  m = RNE(255x) in [0,255]; J = m>>4; r = m&15 (exact fp tricks).
  DVE builds 16-wide one-hots of J and r (bf16 4x tensor_scalar is_equal).
  TensorE: 8 chunks are packed per matmul: stationary = ohJ of 8 chunks side
  by side [128, 128] (full-width -> Fast Weight Load), moving = ohr of the
  same 8 chunks [128, 128]; out[8J+i, 8r+i'] accumulates in PSUM.  Only the
  slot-diagonal i=i' sub-blocks are meaningful (extracted on host); packing
  cuts 768 small matmuls down to 96 full-width ones.
"""
import sys
if "/opt/trn_rl_repo" not in sys.path:
    sys.path.insert(0, "/opt/trn_rl_repo")

import numpy as np
from concourse import bacc, tile
from concourse.bass_utils import run_bass_kernel_spmd
import concourse.mybir as mybir

# ---- problem constants (hardcoded per spec) ----
B, C, H, W = 4, 3, 256, 256
UNITS = B * C                  # 12 independent histograms
NPIX = H * W                   # 65536 pixels per unit
NCORES = 8
PIX_PER_CORE = NPIX // NCORES  # 8192 pixels per unit per core
CHUNKS_PER_UNIT = PIX_PER_CORE // 128  # 64
NCHUNK = UNITS * CHUNKS_PER_UNIT       # 768 chunks of 128 pixels
SIGMA = 0.01
BINS = 256
SIG_B = 255.0 * SIGMA                  # 2.55 bins: gaussian width in bin units
KTAP = 16                              # host conv halfwidth (g(16/2.55) ~ 6e-18)
NGROUP = 3                             # pipeline groups (4 units each)
GCHUNK = NCHUNK // NGROUP              # 256 chunks per group
PACK = 8                               # chunks per matmul
DT = mybir.dt

_COMPILED = None


def _emit_body(nc, tc, pool, pipe, psum_pool, x_ext, tbl_ext,
               emit_cols=True, emit_mm=True):
    xc = pool.tile([128, NCHUNK], DT.float32)
    nc.sync.dma_start(xc[:], x_ext[:])

    # slot-major content: column i*96 + gp holds chunk (slot i, global pack gp)
    m_i = pool.tile([128, NCHUNK], DT.int32)
    m_f = pool.tile([128, NCHUNK], DT.float32)
    J_i = pool.tile([128, NCHUNK], DT.int32)
    J_f = pool.tile([128, NCHUNK], DT.float32)
    J_s = pool.tile([128, NCHUNK], DT.bfloat16)
    r_s = pool.tile([128, NCHUNK], DT.bfloat16)

    # m = RNE(255*x) via int32-convert; exact fp32 (255x <= 255).
    nc.vector.tensor_scalar(m_i[:], xc[:], 255.0, None, mybir.AluOpType.mult)
    nc.vector.tensor_copy(m_f[:], m_i[:])
    # J = floor(m/16) = RNE(m/16 - 15/32): m/16 lies on a 1/16 grid, so the
    # offset keeps every value >= 1/32 away from a rounding boundary.
    nc.vector.tensor_scalar(J_i[:], m_f[:], 0.0625, -0.46875,
                            mybir.AluOpType.mult, mybir.AluOpType.add)
    nc.vector.tensor_copy(J_f[:], J_i[:])
    nc.vector.tensor_copy(J_s[:], J_i[:])
    # r = m - 16J in [0,16)
    nc.vector.scalar_tensor_tensor(r_s[:], J_f[:], -16.0, m_f[:],
                                   mybir.AluOpType.mult, mybir.AluOpType.add)

    accs = [psum_pool.tile([128, 4 * PACK * 16], DT.float32, name=f"acc{g}")
            for g in range(NGROUP)] if emit_mm else None
    out_sb = pool.tile([128, NGROUP * 512], DT.float32)

    NPACKS = NCHUNK // PACK  # 96 global packs
    # one-hots in (v, slot, pack) layout: ohJ[:, v, i, gp] = [J == v] for
    # chunk (slot i, global pack gp).  Each is_equal reads/writes a fully
    # contiguous [128, 768] region -> DVE 4x mode (~240ns/op measured).  The
    # matmul stationary slice [:, :, :, gp] has free strides (768, 96) which
    # collapse to a single free dim of stride 96 (legal weights AP).
    ohJ = pipe.tile([128, 16, PACK, NPACKS], DT.bfloat16, tag="ohJ")
    ohr = pipe.tile([128, 16, PACK, NPACKS], DT.bfloat16, tag="ohr")
    FLAT = frozenset({0})  # collapse free dims -> flat AP so DVE 4x engages
    if emit_cols:
        for v in range(16):
            nc.vector.tensor_scalar(ohJ[:, v, :, :].opt(FLAT), J_s[:],
                                    float(v), None, mybir.AluOpType.is_equal)
            nc.vector.tensor_scalar(ohr[:, v, :, :].opt(FLAT), r_s[:],
                                    float(v), None, mybir.AluOpType.is_equal)
    npk = CHUNKS_PER_UNIT // PACK  # 8 packs per unit
    for g in range(NGROUP):
        if emit_mm:
            for uu in range(4):  # 4 units per group
                for q in range(npk):
                    gp = (g * 4 + uu) * npk + q
                    nc.tensor.matmul(accs[g][:, uu * 128:(uu + 1) * 128],
                                     ohJ[:, :, :, gp],
                                     ohr[:, :, :, gp],
                                     start=(q == 0), stop=(q == npk - 1))
            nc.scalar.copy(out_sb[:, g * 512:(g + 1) * 512], accs[g][:])
        else:
            # ablation: keep ACT-copy volume identical without reading PSUM
            nc.scalar.copy(out_sb[:, g * 512:(g + 1) * 512], xc[:, 0:512])
        nc.sync.dma_start(tbl_ext[:, g * 512:(g + 1) * 512],
                          out_sb[:, g * 512:(g + 1) * 512])


def _build(loop_n=1, emit_cols=True, emit_mm=True):
    nc = bacc.Bacc("TRN2", target_bir_lowering=False, debug=False,
                   num_devices=NCORES)
    x_ext = nc.declare_dram_parameter("xc", [128, NCHUNK], DT.float32,
                                      isOutput=False)
    tbl_ext = nc.declare_dram_parameter("table", [128, NGROUP * 512],
                                        DT.float32, isOutput=True)

    with tile.TileContext(nc) as tc:
        with (
            tc.tile_pool(name="pool", bufs=1) as pool,
            tc.tile_pool(name="pipe", bufs=2) as pipe,
            tc.tile_pool(name="psum", bufs=1, space="PSUM") as psum_pool,
        ):
            if loop_n == 1:
                _emit_body(nc, tc, pool, pipe, psum_pool, x_ext, tbl_ext,
                           emit_cols, emit_mm)
            else:
                engs = [mybir.EngineType.PE, mybir.EngineType.DVE,
                        mybir.EngineType.Activation, mybir.EngineType.SP,
                        mybir.EngineType.Pool]
                with tc.For_i(0, loop_n, 1, hint_engines=engs):
                    _emit_body(nc, tc, pool, pipe, psum_pool, x_ext, tbl_ext,
                               emit_cols, emit_mm)

    nc.compile()
    return nc


def _get_compiled():
    global _COMPILED
    if _COMPILED is None:
        _COMPILED = _build()
    return _COMPILED


def _shard_x(x):
    """x (B,C,H,W) -> per-core [128, NCHUNK] arrays in slot-major order:
    column i*96 + u*8 + q holds chunk (unit u, pack q, slot i), whose pixels
    are unit u's core-slice pixels [128*(8q+i) : 128*(8q+i+1)]."""
    xu = np.ascontiguousarray(x.reshape(UNITS, NPIX))
    shards = []
    for core in range(NCORES):
        sl = xu[:, core * PIX_PER_CORE:(core + 1) * PIX_PER_CORE]
        # (u, q, i, p) -> (p, i, u, q)
        sl = sl.reshape(UNITS, CHUNKS_PER_UNIT // PACK, PACK, 128)
        sl = sl.transpose(3, 2, 0, 1)
        shards.append(np.ascontiguousarray(sl.reshape(128, NCHUNK), np.float32))
    return shards


def _postprocess(tables):
    """tables: list of NCORES arrays [128, 1536] -> cdf (B, C, BINS) fp32."""
    cnt = np.zeros((UNITS, 16, 16), np.float64)   # [unit, J, r]
    for t in tables:
        # rows = (J:16, i:8); cols = (g:3, uu:4, r:16, i':8); diag i==i'
        t6 = t.reshape(16, 8, NGROUP, 4, 16, 8).astype(np.float64)
        cnt += np.einsum('jiguri->gujr', t6).reshape(UNITS, 16, 16)
    count = cnt.reshape(UNITS, BINS)              # bin m = 16J + r
    ks = np.arange(-KTAP, KTAP + 1)
    g = np.exp(-(ks / SIG_B) ** 2)
    hist = np.zeros((UNITS, BINS), np.float64)
    for i, k in enumerate(ks):
        lo, hi = max(0, k), min(BINS, BINS + k)
        hist[:, lo:hi] += g[i] * count[:, lo - k:hi - k]
    pdf = hist / (hist.sum(-1, keepdims=True) + 1e-6)
    cdf = np.cumsum(pdf, -1)
    return cdf.reshape(B, C, BINS).astype(np.float32)


def run_device(x, trace=False):
    nc = _get_compiled()
    in_maps = [{"xc": s} for s in _shard_x(np.asarray(x))]
    res = run_bass_kernel_spmd(nc, in_maps, list(range(NCORES)), trace=trace)
    tables = [res.results[i]["table"] for i in range(NCORES)]
    return tables, res


def kernel(x, centers):
    # centers is linspace(0,1,256) by construction; bin geometry is hardcoded.
    tables, _ = run_device(x)
    return _postprocess(tables)


if __name__ == "__main__":
    import jax, jax.numpy as jnp
    key = jax.random.key(0)
    k1, _ = jax.random.split(key)
    x = np.asarray(jax.random.uniform(k1, (B, C, H, W), dtype=jnp.float32))
    centers = np.linspace(0, 1, BINS, dtype=np.float32)
    out = kernel(x, centers)
    print("kernel output", out.shape, out.dtype, out[0, 0, :5], out[0, 0, -1])
